# revision 7
# baseline (speedup 1.0000x reference)
"""Trainium2 Bass kernel for per-class variance loss (segment_reduce).

loss = sum_c sum_d mean_{i: y_i=c} (x_id - mu_cd)^2
     = sum_c ( s2[c] - sum_d class_sum[c,d]^2 / n_c ) / n_c

with   class_sum[c,d] = sum_{i: y_i=c} x[i,d]        (segment sum)
       s2[c]          = sum_{i: y_i=c} sum_d x[i,d]^2 (segment sum of row ssq)
       n_c            = count of class c (safe-clamped to >= 1)

Device work (the 256 MB feature read, data-parallel over 8 cores):
  - class_sum via one-hot matmul on TensorE (PSUM-accumulated, fp32r)
  - row sum-of-squares split 1:1 between ScalarE (fused Square+accum)
    and VectorE (fused scalar_tensor_tensor mult+accum, one pass)
Host work (tiny): one-hot construction from labels, sharding, final
[C,D]-partial reduction across cores and the scalar loss formula.

v2 layout (from trace analysis of the 98.7-103.6us baseline):
  - stream was gapless 9.1->91.3us at ~420 GB/s; the loss was an 8.5us
    serialized drain (DVE backed up: 2-pass mult+reduce) + 3.8us teardown.
  - DVE now does square+reduce in ONE pass via scalar_tensor_tensor
    accum_out (native InstTensorScalarPtr - NOT the custom-uop
    tensor_tensor_reduce, which INTERNAL-faults on this stack).
  - chunks: 512 first (early compute start), 4MB/32KB-line middles
    (higher per-engine DMA rate), 256/128/128 tail (short drain chain).
  - oh rides the Scalar engine's HWDGE queue, in parallel with the x
    stream on Sync's queue; rq ships from Scalar's queue and cs from
    Sync's queue so the two output triggers fire concurrently.
  - cs PSUM->SBUF drain copies split Scalar/Vector (GpSimd copies run
    below roofline; software impl).
"""

import os

import numpy as np

P = 128  # SBUF partitions
D = 1024  # feature dim
C = 10  # num classes
N = 65536  # samples
NCORES = 8
SHARD = N // NCORES  # 8192 rows per core
NTILES = SHARD // P  # 64 row-tiles per core
MM_HALF = 512  # PSUM bank = 512 fp32 per partition

_CHUNK_MODE = os.environ.get("BASS_CHUNKS", "v2")
if _CHUNK_MODE == "v2":
    # 512 head (compute starts ~5us earlier than a 4MB head), 4MB middles
    # (32KB partition lines: ~27 vs 26.3 GB/s per DMA engine), fine tail so
    # the last tile's data lands with minimal residual stream behind it.
    CHUNK_SIZES = [512] + [1024] * 7 + [256, 128, 128]
    _XBUFS_DEFAULT = 5
else:  # "old" baseline layout
    CHUNK_SIZES = [512] * 15 + [128, 128, 128, 128]
    _XBUFS_DEFAULT = 9
assert sum(CHUNK_SIZES) == SHARD
CHUNK_BASES = [sum(CHUNK_SIZES[:i]) for i in range(len(CHUNK_SIZES))]

_PROGRAM = None

# DVE path: fused one-pass square+reduce via scalar_tensor_tensor accum_out
# (default) vs the old two-pass tensor_tensor + tensor_reduce.
_DVE_STT = os.environ.get("BASS_DVE_STT", "1") == "1"
_XBUFS = int(os.environ.get("BASS_XBUFS", str(_XBUFS_DEFAULT)))
# rq ships in two pieces: [0, RQ_SPLIT) mid-stream, the rest at the end.
RQ_SPLIT = int(os.environ.get("BASS_RQ_SPLIT", "60"))


def _build_program():
    import concourse.bacc as bacc
    import concourse.tile as tile
    from concourse import mybir

    f32 = mybir.dt.float32
    f32r = mybir.dt.float32r
    Square = mybir.ActivationFunctionType.Square

    nc = bacc.Bacc(
        "TRN2",
        target_bir_lowering=False,
        debug=False,
        enable_asserts=False,
        num_devices=NCORES,
    )
    # x/oh declared float32r (same bits as fp32) so the class-sum matmuls can
    # run in the PE's fast fp32 mode; walrus requires producer dtype == f32r.
    x_dram = nc.dram_tensor("x", [SHARD, D], f32r, kind="ExternalInput").ap()
    # one-hot, pre-swizzled on host: oh[p, idx*C + k] is the one-hot of the
    # sample living in partition p of global tile idx (chunked layout).
    oh_dram = nc.dram_tensor("oh", [P, NTILES * C], f32r, kind="ExternalInput").ap()
    cs_dram = nc.dram_tensor("cs", [C, D], f32, kind="ExternalOutput").ap()
    # per-sample row sum-of-squares, tile-column layout (host segment-sums).
    # Shipped in two pieces: tiles [0, RQ_SPLIT) mid-stream from the (idle)
    # Sync queue — its 128-descriptor generation (~2.1us at ~16.6ns/desc)
    # happens under the stream — and the tail piece at the end from the
    # Scalar queue, concurrent with the cs ship on Sync.
    rq_dram = nc.dram_tensor("rq", [P, NTILES], f32, kind="ExternalOutput").ap()

    with tile.TileContext(nc) as tc:
        with (
            tc.tile_pool(name="xio", bufs=_XBUFS) as xpool,
            tc.tile_pool(name="persist", bufs=1) as ppool,
            tc.tile_pool(name="sqp", bufs=3) as sqpool,
            tc.tile_pool(name="psum", bufs=1, space="PSUM") as pspool,
        ):
            oh_all = ppool.tile([P, NTILES * C], f32r, name="oh_all")
            rs_all = ppool.tile([P, NTILES], f32, name="rs_all")
            cs_sb = ppool.tile([C, D], f32, name="cs_sb")

            cs_ps0 = pspool.tile([C, MM_HALF], f32, name="cs_ps0")
            cs_ps1 = pspool.tile([C, MM_HALF], f32, name="cs_ps1")

            # oh on the Scalar engine's HWDGE queue: triggers in parallel with
            # the x stream on Sync's queue, lands (~13us) well before the
            # first compute needs it.
            nc.scalar.dma_start(out=oh_all[:], in_=oh_dram[:])

            idx = -1
            for ci, (rows, base) in enumerate(zip(CHUNK_SIZES, CHUNK_BASES)):
                tpc = rows // P
                x_chunk = xpool.tile(
                    [P, tpc * D], f32r, name=f"x_chunk{ci}", tag="x_chunk"
                )
                src = x_dram[base : base + rows, :].rearrange(
                    "(p t) d -> p (t d)", p=P
                )
                nc.sync.dma_start(out=x_chunk[:], in_=src)
                for t in range(tpc):
                    idx += 1
                    first = idx == 0
                    last = idx == NTILES - 1
                    xt = x_chunk[:, t * D : (t + 1) * D]
                    oht = oh_all[:, idx * C : (idx + 1) * C]
                    nc.tensor.matmul(
                        cs_ps0[:],
                        lhsT=oht,
                        rhs=xt[:, 0:MM_HALF],
                        start=first,
                        stop=last,
                    )
                    nc.tensor.matmul(
                        cs_ps1[:],
                        lhsT=oht,
                        rhs=xt[:, MM_HALF:D],
                        start=first,
                        stop=last,
                    )
                    rs = rs_all[:, idx : idx + 1]
                    sq = sqpool.tile([P, D], f32, name=f"sq{idx}", tag="sq")
                    if idx % 2 == 0:
                        # ScalarE: fused Square + accum (1 pass + accum read)
                        nc.scalar.activation(
                            out=sq[:],
                            in_=xt.bitcast(f32),
                            func=Square,
                            accum_out=rs,
                        )
                    elif _DVE_STT:
                        # VectorE: fused (x*1)*x with accum_out — one pass.
                        nc.vector.scalar_tensor_tensor(
                            out=sq[:],
                            in0=xt.bitcast(f32),
                            scalar=1.0,
                            in1=xt.bitcast(f32),
                            op0=mybir.AluOpType.mult,
                            op1=mybir.AluOpType.mult,
                            accum_out=rs,
                        )
                    else:
                        nc.vector.tensor_tensor(
                            out=sq[:],
                            in0=xt.bitcast(f32),
                            in1=xt.bitcast(f32),
                            op=mybir.AluOpType.mult,
                        )
                        nc.vector.tensor_reduce(
                            out=rs,
                            in_=sq[:],
                            axis=mybir.AxisListType.X,
                            op=mybir.AluOpType.add,
                        )
            # rq head: triggered by the Sync engine (idle once the last chunk
            # trigger has fired, ~51us) as soon as tiles [0, RQ_SPLIT) have
            # accumulated (~86us); its 128 descriptors generate under the
            # stream, off the critical tail. Emitted AFTER the chunk loop so
            # its rs-ready wait cannot delay the tail chunks' triggers, which
            # share the Sync queue.
            nc.sync.dma_start(out=rq_dram[:, 0:RQ_SPLIT], in_=rs_all[:, 0:RQ_SPLIT])

            # Drain. rq tail from Scalar's queue, cs from Sync's queue — the
            # two end-of-kernel triggers fire concurrently. PSUM drain copies
            # split Scalar/Vector so each engine does one 512-col copy right
            # after its final square.
            nc.scalar.dma_start(
                out=rq_dram[:, RQ_SPLIT:NTILES], in_=rs_all[:, RQ_SPLIT:NTILES]
            )
            nc.scalar.copy(out=cs_sb[:, 0:MM_HALF], in_=cs_ps0[:])
            nc.vector.tensor_copy(out=cs_sb[:, MM_HALF:D], in_=cs_ps1[:])
            nc.sync.dma_start(out=cs_dram[:], in_=cs_sb[:])

    nc.compile()
    return nc


def _get_program():
    global _PROGRAM
    if _PROGRAM is None:
        _PROGRAM = _build_program()
    return _PROGRAM


def _install_ntff_hook_shim():
    """Make `antenv.axon_hooks` importable so run_bass_kernel_spmd(trace=True)
    can capture NTFF profiles under axon. No-op if it already exists."""
    import ctypes
    import contextlib
    import sys
    import types

    try:
        from antenv.axon_hooks import get_axon_ntff_profile_hook  # noqa: F401

        return
    except ImportError:
        pass

    so_path = "/opt/axon/libaxon_pjrt.so"
    try:
        lib = ctypes.CDLL(so_path)
        if not hasattr(lib, "axon_start_nrt_profile"):
            return
    except OSError:
        return
    lib.axon_start_nrt_profile.argtypes = [
        ctypes.POINTER(ctypes.c_int64),
        ctypes.c_size_t,
    ]
    lib.axon_start_nrt_profile.restype = ctypes.c_int64
    lib.axon_stop_nrt_profile.argtypes = [ctypes.c_char_p]
    lib.axon_stop_nrt_profile.restype = ctypes.c_int64

    @contextlib.contextmanager
    def _hook(output_dir, device_ids):
        import jax

        jax.devices()
        if device_ids:
            ids = (ctypes.c_int64 * len(device_ids))(*device_ids)
            rc = lib.axon_start_nrt_profile(ids, len(device_ids))
        else:
            rc = lib.axon_start_nrt_profile(None, 0)
        if rc != 0:
            raise RuntimeError(f"axon_start_nrt_profile rc={rc}")
        try:
            yield
        finally:
            n = lib.axon_stop_nrt_profile(str(output_dir).encode())
            if n < 0:
                raise RuntimeError(f"axon_stop_nrt_profile rc={n}")

    mod = types.ModuleType("antenv.axon_hooks")
    mod.get_axon_ntff_profile_hook = lambda: _hook
    mod.set_axon_ntff_profile_hook = lambda h: None
    sys.modules["antenv.axon_hooks"] = mod


LAST_RESULT = None  # BassKernelResults of the most recent run (for test.py)


def _swizzle_rows(arr2d):
    """[SHARD, W] row-major -> [P, NTILES*W] in the device tile layout.

    Shard row base + p*tpc + t (within chunk at `base`, tpc tiles) lands at
    [p, (idx0+t)*W : (idx0+t+1)*W] where idx0 is the chunk's first tile index.
    """
    W = arr2d.shape[1]
    out = np.empty((P, NTILES * W), dtype=arr2d.dtype)
    idx0 = 0
    for rows, base in zip(CHUNK_SIZES, CHUNK_BASES):
        tpc = rows // P
        out[:, idx0 * W : (idx0 + tpc) * W] = arr2d[base : base + rows].reshape(
            P, tpc * W
        )
        idx0 += tpc
    return out


def _unswizzle_cols(arr):
    """[P, NTILES] tile-column layout -> [SHARD] row-major (inverse of above)."""
    out = np.empty(SHARD, dtype=arr.dtype)
    idx0 = 0
    for rows, base in zip(CHUNK_SIZES, CHUNK_BASES):
        tpc = rows // P
        out[base : base + rows] = arr[:, idx0 : idx0 + tpc].reshape(rows)
        idx0 += tpc
    return out


def _make_in_maps(x, onehot):
    in_maps = []
    for k in range(NCORES):
        xs = np.ascontiguousarray(x[k * SHARD : (k + 1) * SHARD])
        oh_sw = np.ascontiguousarray(_swizzle_rows(onehot[k * SHARD : (k + 1) * SHARD]))
        in_maps.append({"x": xs, "oh": oh_sw})
    return in_maps


def kernel(flatten_features, data_label):
    global LAST_RESULT
    from concourse import bass_utils

    x = np.asarray(flatten_features, dtype=np.float32)
    labels = np.asarray(data_label).astype(np.int64).reshape(-1)

    counts = np.bincount(labels, minlength=C).astype(np.float64)
    onehot = np.zeros((N, C), dtype=np.float32)
    onehot[np.arange(N), labels] = 1.0

    in_maps = _make_in_maps(x, onehot)
    nc = _get_program()

    trace = os.environ.get("BASS_KERNEL_TRACE") == "1"
    if trace:
        _install_ntff_hook_shim()
        trace_cores = os.environ.get("BASS_KERNEL_TRACE_CORES", "0")
        tc_list = [int(s) for s in trace_cores.split(",") if s != ""]
        res = bass_utils.run_bass_kernel_spmd(
            nc,
            in_maps,
            core_ids=list(range(NCORES)),
            trace=True,
            trace_cores=tc_list,
        )
    else:
        res = bass_utils.run_bass_kernel_spmd(
            nc, in_maps, core_ids=list(range(NCORES))
        )
    LAST_RESULT = res

    cs = np.zeros((C, D), np.float64)
    s2 = np.zeros((C,), np.float64)
    for k, r in enumerate(res.results):
        cs += r["cs"].astype(np.float64)
        rq = _unswizzle_cols(r["rq"]).astype(np.float64)
        lab = labels[k * SHARD : (k + 1) * SHARD]
        s2 += np.bincount(lab, weights=rq, minlength=C)

    safe = np.maximum(counts, 1.0)
    b = (cs**2).sum(axis=1) / safe
    loss = ((s2 - b) / safe).sum()
    return np.array(loss, dtype=np.float32)


# revision 15
# speedup vs baseline: 1.1939x; 1.1939x over previous
"""Trainium2 Bass kernel for per-class variance loss (segment_reduce).

loss = sum_c sum_d mean_{i: y_i=c} (x_id - mu_cd)^2
     = sum_c ( s2[c] - sum_d class_sum[c,d]^2 / n_c ) / n_c

with   class_sum[c,d] = sum_{i: y_i=c} x[i,d]        (segment sum)
       s2[c]          = sum_{i: y_i=c} sum_d x[i,d]^2 (segment sum of row ssq)
       n_c            = count of class c (safe-clamped to >= 1)

Device work (the 256 MB feature read, data-parallel over 8 cores):
  - class_sum via one-hot matmul on TensorE (PSUM-accumulated, fp32r)
  - row sum-of-squares split 1:1 between ScalarE (fused Square+accum)
    and VectorE (fused scalar_tensor_tensor mult+accum, one pass)
Host work (tiny): one-hot construction from labels, sharding, final
[C,D]-partial reduction across cores and the scalar loss formula.

v2 layout (from trace analysis of the 98.7-103.6us baseline):
  - stream was gapless 9.1->91.3us at ~420 GB/s; the loss was an 8.5us
    serialized drain (DVE backed up: 2-pass mult+reduce) + 3.8us teardown.
  - DVE now does square+reduce in ONE pass via scalar_tensor_tensor
    accum_out (native InstTensorScalarPtr - NOT the custom-uop
    tensor_tensor_reduce, which INTERNAL-faults on this stack).
  - chunks: 512 first (early compute start), 4MB/32KB-line middles
    (higher per-engine DMA rate), 256/128/128 tail (short drain chain).
  - oh rides the Scalar engine's HWDGE queue, in parallel with the x
    stream on Sync's queue; rq ships from Scalar's queue and cs from
    Sync's queue so the two output triggers fire concurrently.
  - cs PSUM->SBUF drain copies split Scalar/Vector (GpSimd copies run
    below roofline; software impl).
"""

import os

import numpy as np

P = 128  # SBUF partitions
D = 1024  # feature dim
C = 10  # num classes
N = 65536  # samples
NCORES = 8
SHARD = N // NCORES  # 8192 rows per core
NTILES = SHARD // P  # 64 row-tiles per core
MM_HALF = 512  # PSUM bank = 512 fp32 per partition

_CHUNK_MODE = os.environ.get("BASS_CHUNKS", "v4")
if _CHUNK_MODE == "v4":
    # 2MB chunks (16KB partition lines; the 16 DMA engines run 99% busy at
    # ~26.6 GB/s each regardless of 16KB vs 32KB lines, so bigger chunks buy
    # nothing) with a fine tail so the last tile's data lands with minimal
    # residual stream behind it. 10 bufs: chunk triggers wait on slot reuse
    # (consumed(i-10)), and consumers free slots at ~5.5us/chunk vs 4.7us
    # delivery — 10 slots keeps ~40us of trigger margin so a bad-HAM run
    # can't starve the tail descriptors (the v3 4MB/5-buf layout collapsed
    # +18us exactly this way).
    CHUNK_SIZES = [512] * 15 + [256, 128, 128]
    _XBUFS_DEFAULT = 10
elif _CHUNK_MODE == "v2":
    CHUNK_SIZES = [512] + [1024] * 7 + [256, 128, 128]
    _XBUFS_DEFAULT = 5
else:  # "old" baseline layout
    CHUNK_SIZES = [512] * 15 + [128, 128, 128, 128]
    _XBUFS_DEFAULT = 9
assert sum(CHUNK_SIZES) == SHARD
CHUNK_BASES = [sum(CHUNK_SIZES[:i]) for i in range(len(CHUNK_SIZES))]

_PROGRAM = None

# DVE path: fused one-pass square+reduce via scalar_tensor_tensor accum_out
# (default) vs the old two-pass tensor_tensor + tensor_reduce.
_DVE_STT = os.environ.get("BASS_DVE_STT", "1") == "1"
_XBUFS = int(os.environ.get("BASS_XBUFS", str(_XBUFS_DEFAULT)))
# rq ships in two pieces: [0, RQ_SPLIT) mid-stream, the rest at the end.
RQ_SPLIT = int(os.environ.get("BASS_RQ_SPLIT", "60"))


def _build_program():
    import concourse.bacc as bacc
    import concourse.tile as tile
    from concourse import mybir

    f32 = mybir.dt.float32
    f32r = mybir.dt.float32r
    Square = mybir.ActivationFunctionType.Square

    nc = bacc.Bacc(
        "TRN2",
        target_bir_lowering=False,
        debug=False,
        enable_asserts=False,
        num_devices=NCORES,
    )
    # x/oh declared float32r (same bits as fp32) so the class-sum matmuls can
    # run in the PE's fast fp32 mode; walrus requires producer dtype == f32r.
    x_dram = nc.dram_tensor("x", [SHARD, D], f32r, kind="ExternalInput").ap()
    # one-hot, pre-swizzled on host: oh[p, idx*C + k] is the one-hot of the
    # sample living in partition p of global tile idx (chunked layout).
    oh_dram = nc.dram_tensor("oh", [P, NTILES * C], f32r, kind="ExternalInput").ap()
    # cs output carries D class-sum columns + (NTILES - RQ_SPLIT) per-class
    # segment-sums of the tail tiles' row-ssq (computed on PE, see below), so
    # the end-of-kernel ship is ONE 10-descriptor DMA.
    TAIL = NTILES - RQ_SPLIT
    cs_dram = nc.dram_tensor("cs", [C, D + TAIL], f32, kind="ExternalOutput").ap()
    # per-sample row sum-of-squares for tiles [0, RQ_SPLIT), tile-column
    # layout (host segment-sums). Shipped mid-stream from the (by then idle)
    # Sync queue: a [P, n] SBUF-source DMA always needs 128 descriptors
    # (~2.1us generation at ~16.6ns/desc), which must stay off the critical
    # tail — the tail tiles' ssq instead rides the cs ship via PE.
    rq_dram = nc.dram_tensor("rq", [P, RQ_SPLIT], f32, kind="ExternalOutput").ap()

    with tile.TileContext(nc) as tc:
        with (
            tc.tile_pool(name="xio", bufs=_XBUFS) as xpool,
            tc.tile_pool(name="persist", bufs=1) as ppool,
            tc.tile_pool(name="sqp", bufs=4) as sqpool,
            tc.tile_pool(name="psum", bufs=1, space="PSUM") as pspool,
        ):
            oh_all = ppool.tile([P, NTILES * C], f32r, name="oh_all")
            rs_all = ppool.tile([P, NTILES], f32, name="rs_all")
            cs_sb = ppool.tile([C, D + TAIL], f32, name="cs_sb")

            cs_ps0 = pspool.tile([C, MM_HALF], f32, name="cs_ps0")
            cs_ps1 = pspool.tile([C, MM_HALF], f32, name="cs_ps1")
            s2t_ps = pspool.tile([C, TAIL], f32, name="s2t_ps")

            # oh on the Scalar engine's HWDGE queue: triggers in parallel with
            # the x stream on Sync's queue, lands (~13us) well before the
            # first compute needs it.
            nc.scalar.dma_start(out=oh_all[:], in_=oh_dram[:])

            idx = -1
            for ci, (rows, base) in enumerate(zip(CHUNK_SIZES, CHUNK_BASES)):
                tpc = rows // P
                x_chunk = xpool.tile(
                    [P, tpc * D], f32r, name=f"x_chunk{ci}", tag="x_chunk"
                )
                src = x_dram[base : base + rows, :].rearrange(
                    "(p t) d -> p (t d)", p=P
                )
                nc.sync.dma_start(out=x_chunk[:], in_=src)
                for t in range(tpc):
                    idx += 1
                    first = idx == 0
                    last = idx == NTILES - 1
                    xt = x_chunk[:, t * D : (t + 1) * D]
                    oht = oh_all[:, idx * C : (idx + 1) * C]
                    nc.tensor.matmul(
                        cs_ps0[:],
                        lhsT=oht,
                        rhs=xt[:, 0:MM_HALF],
                        start=first,
                        stop=last,
                    )
                    nc.tensor.matmul(
                        cs_ps1[:],
                        lhsT=oht,
                        rhs=xt[:, MM_HALF:D],
                        start=first,
                        stop=last,
                    )
                    rs = rs_all[:, idx : idx + 1]
                    sq = sqpool.tile([P, D], f32, name=f"sq{idx}", tag="sq")
                    if idx % 2 == 0:
                        # ScalarE: fused Square + accum (1 pass + accum read)
                        nc.scalar.activation(
                            out=sq[:],
                            in_=xt.bitcast(f32),
                            func=Square,
                            accum_out=rs,
                        )
                    elif _DVE_STT:
                        # VectorE: fused (x*1)*x with accum_out — one pass.
                        nc.vector.scalar_tensor_tensor(
                            out=sq[:],
                            in0=xt.bitcast(f32),
                            scalar=1.0,
                            in1=xt.bitcast(f32),
                            op0=mybir.AluOpType.mult,
                            op1=mybir.AluOpType.mult,
                            accum_out=rs,
                        )
                    else:
                        nc.vector.tensor_tensor(
                            out=sq[:],
                            in0=xt.bitcast(f32),
                            in1=xt.bitcast(f32),
                            op=mybir.AluOpType.mult,
                        )
                        nc.vector.tensor_reduce(
                            out=rs,
                            in_=sq[:],
                            axis=mybir.AxisListType.X,
                            op=mybir.AluOpType.add,
                        )
            # rq head: triggered by the Sync engine (idle once the last chunk
            # trigger has fired) as soon as tiles [0, RQ_SPLIT) have
            # accumulated (~89us); its 128 descriptors generate under the
            # stream tail, off the critical path. Emitted AFTER the chunk
            # loop so its rs-ready wait cannot delay the tail chunks'
            # triggers, which share the Sync queue (emitting it mid-loop
            # measured +18us for exactly that reason).
            nc.sync.dma_start(out=rq_dram[:], in_=rs_all[:, 0:RQ_SPLIT])

            # Tail tiles' ssq segment-sum on PE: s2t[c, j] =
            # sum_p oh[p, idx_j*C + c] * rs[p, idx_j] — the per-class
            # bincount contribution of tile idx_j, one tiny N=1 matmul each.
            # This keeps the [P, *] 128-descriptor rq-tail ship off the
            # drain; the result rides the cs ship (10 descriptors).
            for j in range(TAIL):
                idx_j = RQ_SPLIT + j
                # plain-fp32 matmul (f32 views): rs is produced as f32 by the
                # accumulators, and walrus rejects f32-produced inputs to an
                # FP32r matmul ("not rounded to FP32r"). N=1, so the 4x
                # slower fp32 PE mode costs nothing here.
                nc.tensor.matmul(
                    s2t_ps[:, j : j + 1],
                    lhsT=oh_all[:, idx_j * C : (idx_j + 1) * C].bitcast(f32),
                    rhs=rs_all[:, idx_j : idx_j + 1],
                    start=True,
                    stop=True,
                )

            # Drain: PSUM copies split Scalar/Vector so each engine does one
            # 512-col copy right after its final square, then ONE cs ship.
            nc.scalar.copy(out=cs_sb[:, 0:MM_HALF], in_=cs_ps0[:])
            nc.scalar.copy(out=cs_sb[:, D : D + TAIL], in_=s2t_ps[:])
            nc.vector.tensor_copy(out=cs_sb[:, MM_HALF:D], in_=cs_ps1[:])
            nc.sync.dma_start(out=cs_dram[:], in_=cs_sb[:])

    nc.compile()
    return nc


def _get_program():
    global _PROGRAM
    if _PROGRAM is None:
        _PROGRAM = _build_program()
    return _PROGRAM


def _install_ntff_hook_shim():
    """Make `antenv.axon_hooks` importable so run_bass_kernel_spmd(trace=True)
    can capture NTFF profiles under axon. No-op if it already exists."""
    import ctypes
    import contextlib
    import sys
    import types

    try:
        from antenv.axon_hooks import get_axon_ntff_profile_hook  # noqa: F401

        return
    except ImportError:
        pass

    so_path = "/opt/axon/libaxon_pjrt.so"
    try:
        lib = ctypes.CDLL(so_path)
        if not hasattr(lib, "axon_start_nrt_profile"):
            return
    except OSError:
        return
    lib.axon_start_nrt_profile.argtypes = [
        ctypes.POINTER(ctypes.c_int64),
        ctypes.c_size_t,
    ]
    lib.axon_start_nrt_profile.restype = ctypes.c_int64
    lib.axon_stop_nrt_profile.argtypes = [ctypes.c_char_p]
    lib.axon_stop_nrt_profile.restype = ctypes.c_int64

    @contextlib.contextmanager
    def _hook(output_dir, device_ids):
        import jax

        jax.devices()
        if device_ids:
            ids = (ctypes.c_int64 * len(device_ids))(*device_ids)
            rc = lib.axon_start_nrt_profile(ids, len(device_ids))
        else:
            rc = lib.axon_start_nrt_profile(None, 0)
        if rc != 0:
            raise RuntimeError(f"axon_start_nrt_profile rc={rc}")
        try:
            yield
        finally:
            n = lib.axon_stop_nrt_profile(str(output_dir).encode())
            if n < 0:
                raise RuntimeError(f"axon_stop_nrt_profile rc={n}")

    mod = types.ModuleType("antenv.axon_hooks")
    mod.get_axon_ntff_profile_hook = lambda: _hook
    mod.set_axon_ntff_profile_hook = lambda h: None
    sys.modules["antenv.axon_hooks"] = mod


LAST_RESULT = None  # BassKernelResults of the most recent run (for test.py)


def _swizzle_rows(arr2d):
    """[SHARD, W] row-major -> [P, NTILES*W] in the device tile layout.

    Shard row base + p*tpc + t (within chunk at `base`, tpc tiles) lands at
    [p, (idx0+t)*W : (idx0+t+1)*W] where idx0 is the chunk's first tile index.
    """
    W = arr2d.shape[1]
    out = np.empty((P, NTILES * W), dtype=arr2d.dtype)
    idx0 = 0
    for rows, base in zip(CHUNK_SIZES, CHUNK_BASES):
        tpc = rows // P
        out[:, idx0 * W : (idx0 + tpc) * W] = arr2d[base : base + rows].reshape(
            P, tpc * W
        )
        idx0 += tpc
    return out


def _unswizzle_cols(arr):
    """[P, ntiles] tile-column layout -> row-major (inverse of above).

    Handles a prefix of tiles (arr may have fewer than NTILES columns,
    provided it ends on a chunk boundary).
    """
    ntiles = arr.shape[1]
    out = np.empty(ntiles * P, dtype=arr.dtype)
    idx0 = 0
    for rows, base in zip(CHUNK_SIZES, CHUNK_BASES):
        tpc = rows // P
        if idx0 + tpc > ntiles:
            break
        out[base : base + rows] = arr[:, idx0 : idx0 + tpc].reshape(rows)
        idx0 += tpc
    assert idx0 == ntiles, "rq piece must end on a chunk boundary"
    return out


def _make_in_maps(x, onehot):
    in_maps = []
    for k in range(NCORES):
        xs = np.ascontiguousarray(x[k * SHARD : (k + 1) * SHARD])
        oh_sw = np.ascontiguousarray(_swizzle_rows(onehot[k * SHARD : (k + 1) * SHARD]))
        in_maps.append({"x": xs, "oh": oh_sw})
    return in_maps


def kernel(flatten_features, data_label):
    global LAST_RESULT
    from concourse import bass_utils

    x = np.asarray(flatten_features, dtype=np.float32)
    labels = np.asarray(data_label).astype(np.int64).reshape(-1)

    counts = np.bincount(labels, minlength=C).astype(np.float64)
    onehot = np.zeros((N, C), dtype=np.float32)
    onehot[np.arange(N), labels] = 1.0

    in_maps = _make_in_maps(x, onehot)
    nc = _get_program()

    trace = os.environ.get("BASS_KERNEL_TRACE") == "1"
    if trace:
        _install_ntff_hook_shim()
        trace_cores = os.environ.get("BASS_KERNEL_TRACE_CORES", "0")
        tc_list = [int(s) for s in trace_cores.split(",") if s != ""]
        res = bass_utils.run_bass_kernel_spmd(
            nc,
            in_maps,
            core_ids=list(range(NCORES)),
            trace=True,
            trace_cores=tc_list,
        )
    else:
        res = bass_utils.run_bass_kernel_spmd(
            nc, in_maps, core_ids=list(range(NCORES))
        )
    LAST_RESULT = res

    cs = np.zeros((C, D), np.float64)
    s2 = np.zeros((C,), np.float64)
    head_rows = RQ_SPLIT * P  # rows covered by the rq head piece
    for k, r in enumerate(res.results):
        cs += r["cs"][:, 0:D].astype(np.float64)
        # tail tiles' per-class ssq, segment-summed on PE
        s2 += r["cs"][:, D:].astype(np.float64).sum(axis=1)
        rq = _unswizzle_cols(r["rq"]).astype(np.float64)
        lab = labels[k * SHARD : k * SHARD + head_rows]
        s2 += np.bincount(lab, weights=rq, minlength=C)

    safe = np.maximum(counts, 1.0)
    b = (cs**2).sum(axis=1) / safe
    loss = ((s2 - b) / safe).sum()
    return np.array(loss, dtype=np.float32)
